# revision 22
# baseline (speedup 1.0000x reference)
"""GAT layer kernel for 8 TRN2 NeuronCores (self-contained).

Sharding: core c handles batch b = c//2 and head-pair (2*(c%2), 2*(c%2)+1).
Each core computes O_h = softmax_num(leaky(s_i+s_j)+bias) @ [t_h | 1] for its
two heads (unnormalized numerator + row-sum Z in column 256); the host
normalizes by 1/(4Z), sums the two cores per batch and adds mean(b).

Device pipeline per unit (i-tile, head), software-pipelined 3 deep so the
PE never idles (HAM stays at K=8/8):
  S0: PE   PSUM_A(half) = rank-4(w = s_i + s_j, split-fp16) + I @ bias
      ACT/DVE  L = leaky(PSUM_A)  (Prelu on ACT for 2/3 units, STT on DVE)
      DVE  nm = -rowmax(L)
  S1: ACT  E = Exp(L + nm) -> bf16 halves
      DMA  transpose E halves -> ET (j on partitions)
  S2: PE   O[128,257] += ET_J^T.T @ [t_J | 1]   (col 256 = row-sum Z)
      DVE  copy O -> SBUF;  DMA -> DRAM
"""
import numpy as np
import ml_dtypes

B, N, F_IN, F_OUT, H = 4, 2048, 256, 256, 4
P = 128
NT = N // P  # 16
FO1 = F_OUT + 1  # 257: t tiles carry a ones-column so O[:,256] = Z

_NC = None


def _build():
    import concourse.tile as tile
    from concourse import bacc, mybir

    dt = mybir.dt
    f32, f16, bf16 = dt.float32, dt.float16, dt.bfloat16
    AF = mybir.ActivationFunctionType
    ALU = mybir.AluOpType

    nc = bacc.Bacc("TRN2", target_bir_lowering=False, debug=False, num_devices=8)

    d_xhi = nc.dram_tensor("xhi", [F_IN, N], f16, kind="ExternalInput").ap()
    d_xlo = nc.dram_tensor("xlo", [F_IN, N], f16, kind="ExternalInput").ap()
    d_w = nc.dram_tensor("w", [2 * F_IN, F_OUT], f16, kind="ExternalInput").ap()
    d_wa = nc.dram_tensor("wa", [P, 8], f16, kind="ExternalInput").ap()
    d_ba = nc.dram_tensor("ba", [1, 2], f32, kind="ExternalInput").ap()
    d_ident = nc.dram_tensor("ident", [P, P], bf16, kind="ExternalInput").ap()
    d_bias = nc.dram_tensor("biasm", [N, N], bf16, kind="ExternalInput").ap()
    d_out = nc.dram_tensor("out", [2 * N, FO1], f32, kind="ExternalOutput").ap()

    NU = 2 * NT  # 32 units: u -> (I, h) = (u >> 1, u & 1)

    with tile.TileContext(nc) as tc:
        with tc.tile_pool(name="constp", bufs=1) as constp, \
             tc.tile_pool(name="xpool", bufs=1) as xpool, \
             tc.tile_pool(name="tpool", bufs=1) as tpool, \
             tc.tile_pool(name="rowp", bufs=1) as rowp, \
             tc.tile_pool(name="work", bufs=1) as work:

            ident = constp.tile([P, P], bf16)
            nc.sync.dma_start(ident[:], d_ident[:])
            ones_row = constp.tile([1, N], f16)
            nc.gpsimd.memset(ones_row[:], 1.0)
            wa_sb = constp.tile([P, 8], f16)
            nc.sync.dma_start(wa_sb[:], d_wa[:])
            ba_sb = constp.tile([1, 2], f32)
            nc.sync.dma_start(ba_sb[:], d_ba[:])

            xhi = [xpool.tile([P, N], f16, name=f"xhi{c}") for c in range(2)]
            xlo = [xpool.tile([P, N], f16, name=f"xlo{c}") for c in range(2)]
            for c in range(2):
                nc.sync.dma_start(xhi[c][:], d_xhi[c * P:(c + 1) * P, :])
                nc.sync.dma_start(xlo[c][:], d_xlo[c * P:(c + 1) * P, :])
            wsb = [[xpool.tile([P, F_OUT], f16, name=f"wsb{h}_{c}") for c in range(2)]
                   for h in range(2)]
            for h in range(2):
                for c in range(2):
                    nc.sync.dma_start(wsb[h][c][:],
                                      d_w[h * F_IN + c * P: h * F_IN + (c + 1) * P, :])

            t_tiles = [[tpool.tile([P, FO1], bf16, name=f"t{h}_{J}") for J in range(NT)]
                       for h in range(2)]
            LT4 = [rowp.tile([4, N], f16, name=f"LT4_{h}") for h in range(2)]
            RT4 = [rowp.tile([4, N], f16, name=f"RT4_{h}") for h in range(2)]

            with tc.tile_pool(name="pss", bufs=2, space="PSUM") as pss:
                for h in range(2):
                    # ---- s = x @ (W a) + b.a  (split-fp16, M=1 matmuls) ----
                    # s computed on one partition, then DMA-spread to [4, 512]
                    # so the hi/lo splits run on 4 lanes instead of 1.
                    s_row = rowp.tile([1, N], f32, name=f"s_row{h}")
                    s_row4 = rowp.tile([4, 512], f32, name=f"s_row4{h}")
                    for q in range(4):
                        sl = slice(q * 512, (q + 1) * 512)
                        s_ps = pss.tile([1, 512], f32, name=f"s_ps{h}_{q}", tag="s_ps")
                        pieces = [(0, 0), (0, 1), (1, 0)]  # (x split, wa split)
                        n_mm = len(pieces) * 2
                        k = 0
                        for xs, ws in pieces:
                            xt = xhi if xs == 0 else xlo
                            for kc in range(2):
                                col = h * 4 + ws * 2 + kc
                                nc.tensor.matmul(
                                    s_ps[0:1, :], wa_sb[:, col:col + 1], xt[kc][:, sl],
                                    start=(k == 0), stop=(k == n_mm - 1))
                                k += 1
                        nc.scalar.activation(s_row[0:1, sl], s_ps[0:1, :],
                                             AF.Identity,
                                             bias=ba_sb[0:1, h:h + 1], scale=1.0)
                        nc.sync.dma_start(s_row4[q:q + 1, :], s_row[0:1, sl])
                    s_hi4 = rowp.tile([4, 512], f16, name=f"s_hi4{h}")
                    nc.scalar.activation(s_hi4[:], s_row4[:], AF.Identity)
                    s_rem4 = rowp.tile([4, 512], f32, name=f"s_rem4{h}")
                    nc.vector.tensor_sub(s_rem4[:], s_row4[:], s_hi4[:])
                    s_lo4 = rowp.tile([4, 512], f16, name=f"s_lo4{h}")
                    nc.scalar.activation(s_lo4[:], s_rem4[:], AF.Identity)

                    nc.gpsimd.memset(LT4[h][:], 1.0)
                    nc.gpsimd.memset(RT4[h][:], 1.0)
                    for q in range(4):
                        qs = slice(q * 512, (q + 1) * 512)
                        nc.sync.dma_start(LT4[h][0:1, qs], s_hi4[q:q + 1, :])
                        nc.sync.dma_start(LT4[h][1:2, qs], s_lo4[q:q + 1, :])
                        nc.sync.dma_start(RT4[h][2:3, qs], s_hi4[q:q + 1, :])
                        nc.sync.dma_start(RT4[h][3:4, qs], s_lo4[q:q + 1, :])

                    # ---- t_h = x_hi @ W_h (node-major bf16 tiles, ones col) ----
                    for J in range(NT):
                        t_ps = pss.tile([P, F_OUT], f32, name=f"t_ps{h}_{J}", tag="t_ps")
                        jsl = slice(J * P, (J + 1) * P)
                        nc.tensor.matmul(t_ps[:], xhi[0][:, jsl], wsb[h][0][:],
                                         start=True, stop=False)
                        nc.tensor.matmul(t_ps[:], xhi[1][:, jsl], wsb[h][1][:],
                                         start=False, stop=True)
                        nc.vector.tensor_copy(t_tiles[h][J][:, 0:F_OUT], t_ps[:])
                        nc.gpsimd.memset(t_tiles[h][J][:, F_OUT:FO1], 1.0)

            # ---- main softmax/E@t pipeline over units u = (I, h) ----
            # stage state per unit, indexed u % depth
            Lbuf = {}    # u -> L tile [128, 2048] f32
            nmbuf = {}   # u -> nm tile [128, 1] f32
            ETbuf = {}   # u -> (ET half0, ET half1) bf16
            btils = {}   # I -> (btile half0, btile half1) bf16
            Hq = N // 2  # 1024

            def load_btile(I):
                if I >= NT:
                    return
                t0 = work.tile([P, Hq], bf16, name=f"bt{I}_0", tag="bt", bufs=8)
                t1 = work.tile([P, Hq], bf16, name=f"bt{I}_1", tag="bt", bufs=8)
                isl = slice(I * P, (I + 1) * P)
                nc.gpsimd.dma_start(t0[:], d_bias[isl, 0:Hq])
                nc.gpsimd.dma_start(t1[:], d_bias[isl, Hq:N])
                btils[I] = (t0, t1)

            with tc.tile_pool(name="psA", bufs=1, space="PSUM") as psA, \
                 tc.tile_pool(name="psO", bufs=1, space="PSUM") as psO:

                def stage0(u):
                    I, h = u >> 1, u & 1
                    if h == 0:
                        load_btile(I + 2)
                    isl = slice(I * P, (I + 1) * P)
                    L = work.tile([P, N], f32, name=f"L{u}", tag="L", bufs=5)
                    for q in range(2):
                        A = psA.tile([P, Hq], f32, name=f"A{u}_{q}", tag="A", bufs=3)
                        for c in range(2):
                            jsl = slice(q * Hq + c * 512, q * Hq + (c + 1) * 512)
                            nc.tensor.matmul(A[:, c * 512:(c + 1) * 512],
                                             LT4[h][:, isl], RT4[h][:, jsl],
                                             start=True, stop=False)
                        bt = btils[I][q]
                        for c in range(2):
                            nc.tensor.matmul(A[:, c * 512:(c + 1) * 512],
                                             ident[:], bt[:, c * 512:(c + 1) * 512],
                                             start=False, stop=True)
                        lsl = slice(q * Hq, (q + 1) * Hq)
                        if q == 0:
                            nc.scalar.activation(L[:, lsl], A[:], AF.Prelu,
                                                 bias=0.0, scale=1.0, alpha=0.2)
                        else:
                            # split half1: 3/4 of leaky on ACT, 1/4 on DVE
                            # (DVE leaky = 0.2*A + 0.8*relu(A); one PSUM input
                            # per DVE instruction) to balance engine load.
                            nc.scalar.activation(L[:, q * Hq:q * Hq + 512],
                                                 A[:, 0:512], AF.Prelu,
                                                 bias=0.0, scale=1.0, alpha=0.2)
                            Lp = work.tile([P, 512], f32, name=f"Lp{u}",
                                           tag="Lp", bufs=4)
                            nc.vector.tensor_scalar(Lp[:], A[:, 512:Hq], 0.0, 0.8,
                                                    op0=ALU.max, op1=ALU.mult)
                            nc.vector.scalar_tensor_tensor(
                                L[:, q * Hq + 512:N], A[:, 512:Hq], 0.2, Lp[:],
                                op0=ALU.mult, op1=ALU.add)
                    nm = work.tile([P, 1], f32, name=f"nm{u}", tag="nm", bufs=8)
                    nc.vector.tensor_reduce(nm[:], L[:], axis=mybir.AxisListType.X,
                                            op=ALU.max, negate=True)
                    Lbuf[u] = L
                    nmbuf[u] = nm

                def stage1(u):
                    L, nm = Lbuf.pop(u), nmbuf.pop(u)
                    E = work.tile([P, N], bf16, name=f"E{u}", tag="E", bufs=4)
                    nc.scalar.activation(E[:], L[:], AF.Exp, bias=nm[:], scale=1.0)
                    ET = work.tile([P, N], bf16, name=f"ET{u}", tag="ET", bufs=6)
                    et3 = ET[:].rearrange("p (J f) -> p J f", f=P)
                    nc.sync.dma_start_transpose(et3, E[:])
                    ETbuf[u] = ET

                def stage2(u):
                    I, h = u >> 1, u & 1
                    ET = ETbuf.pop(u)
                    O = psO.tile([P, FO1], f32, name=f"O{u}", tag="O", bufs=2)
                    for J in range(NT):
                        nc.tensor.matmul(O[:], ET[:, J * P:(J + 1) * P],
                                         t_tiles[h][J][:],
                                         start=(J == 0), stop=(J == NT - 1))
                    Osb = work.tile([P, FO1], f32, name=f"Osb{u}", tag="Osb", bufs=4)
                    nc.vector.tensor_copy(Osb[:], O[:])
                    nc.gpsimd.dma_start(d_out[h * N + I * P: h * N + (I + 1) * P, :],
                                        Osb[:])

                load_btile(0)
                load_btile(1)
                LAG = 4  # E@t trails stage0 by LAG units so PE never stalls
                for v in range(NU + LAG):
                    if v < NU:
                        stage0(v)
                    if 1 <= v <= NU:
                        stage1(v - 1)
                    if v >= LAG:
                        stage2(v - LAG)

    nc.compile()
    return nc


def prepare_in_maps(inputs, bias, W, a, b):
    inputs = np.asarray(inputs, dtype=np.float32)
    bias = np.asarray(bias, dtype=np.float32)
    W = np.asarray(W, dtype=np.float32)
    a = np.asarray(a, dtype=np.float32)
    b = np.asarray(b, dtype=np.float32)

    ident = np.eye(P, dtype=ml_dtypes.bfloat16)
    in_maps = []
    for c in range(8):
        bb = c // 2
        hp = c % 2
        hs = [2 * hp, 2 * hp + 1]
        xT = np.ascontiguousarray(inputs[bb].T)            # [F_IN, N] f32
        xhi = xT.astype(np.float16)
        xlo = (xT - xhi.astype(np.float32)).astype(np.float16)
        Wp = np.concatenate([W[hs[0]], W[hs[1]]], axis=0).astype(np.float16)
        Wa = np.einsum('hfo,ho->hf', W[hs].astype(np.float64),
                       a[hs].astype(np.float64))           # [2, F_IN]
        Wahi = Wa.astype(np.float16)
        Walo = (Wa - Wahi.astype(np.float64)).astype(np.float16)
        wa_pack = np.zeros((P, 8), np.float16)
        for h in range(2):
            for s_, arr in enumerate([Wahi, Walo]):
                for kc in range(2):
                    wa_pack[:, h * 4 + s_ * 2 + kc] = arr[h, kc * P:(kc + 1) * P]
        ba = np.array([[float(np.dot(b[hs[0]].astype(np.float64), a[hs[0]])),
                        float(np.dot(b[hs[1]].astype(np.float64), a[hs[1]]))]],
                      np.float32)
        biasm = bias[bb].astype(ml_dtypes.bfloat16)
        in_maps.append(dict(xhi=xhi, xlo=xlo, w=Wp, wa=wa_pack,
                            ba=ba, ident=ident, biasm=biasm))
    return in_maps


def gather_output(results, b):
    mean_b = np.asarray(b, np.float64).mean(axis=0)        # [F_OUT]
    out = np.zeros((B, N, F_OUT), np.float64)
    for c in range(8):
        bb = c // 2
        O = np.asarray(results[c]["out"], np.float64)      # [2N, 257]
        for h in range(2):
            blk = O[h * N:(h + 1) * N]
            Z = blk[:, F_OUT:F_OUT + 1]
            out[bb] += blk[:, :F_OUT] / (4.0 * Z)
    out += mean_b[None, None, :]
    return out.astype(np.float32)


def get_nc():
    global _NC
    if _NC is None:
        _NC = _build()
    return _NC


_LAST_EXEC_NS = None
_LAST_TRACE = None


def kernel(inputs, bias, W, a, b):
    global _LAST_EXEC_NS, _LAST_TRACE
    from concourse.bass_utils import run_bass_kernel_spmd
    nc = get_nc()
    in_maps = prepare_in_maps(inputs, bias, W, a, b)
    res = run_bass_kernel_spmd(nc, in_maps, core_ids=list(range(8)))
    _LAST_EXEC_NS = res.exec_time_ns
    _LAST_TRACE = res.instructions_and_trace
    return gather_output(res.results, b)
